# revision 11
# baseline (speedup 1.0000x reference)
# LoRA-MoE QK kernel for 8x Trainium2 NeuronCores (Bass/Tile).
#
# Reference computation:
#   routing = softmax(mean(x[:, 611:-1, :]) @ router_W.T + router_b)   [B, E]
#   base    = x @ W.T + b
#   lora    = einsum('bsd,erd->bser', x, A) -> *B,routing -> [B,S,O] * 2.0
#   out     = base + lora
#
# Sharding: data-parallel over the 8192 tokens (1024/core; each core's tokens
# belong to exactly one batch; a batch spans cores {2b, 2b+1}).  Weights
# replicated, host-prepped; router computed on host.
#
# v6 (530us baseline -> 495us swizzle/startup -> 454us NK8=6 -> 437us):
#  - Host-swizzled SBUF images for all inputs: the DMA engines process one
#    descriptor per partition row (~85ns each), so strided narrow loads are
#    descriptor-rate bound.  Images make every copy 2-32KB-contiguous per
#    row; kernel descriptor count drops ~56k -> ~12k and the first matmul
#    starts at ~11us instead of 38us.
#  - DMA issue order == consumption order; panel 0 is consumed k-outer with
#    6 concurrent token PSUM groups + 2 LoRA-t groups (all 8 banks).
#  - PE warm-up: ~20 dummy matmuls on a zeroed SBUF tile run while the first
#    DMAs land, so the HAM clock-gate is already at 8/8 (2.4GHz) when real
#    matmuls start (saves the ~44-matmul cold ramp, ~6us).
#  - Partial fp8: the last NK8=8 of 32 k-tiles of the base matmul run as
#    e4m3 DoubleRow pairs (4 DR MMs replace 8 bf16 MMs per PSUM group),
#    saving ~37us of PE time.  Error is host-side quantization noise and
#    scales as sqrt(NK8): measured 0.0129/0.0157/0.0181 at NK8=4/6/8 vs
#    the 0.02 gate (deterministic for the fixed test seed).
#  - fp16 output (halves output traffic; values are O(10)).

import numpy as np
import ml_dtypes

BF16 = ml_dtypes.bfloat16
E4M3 = ml_dtypes.float8_e4m3fn

B_, S, D, O, E, R = 4, 2048, 4096, 4096, 8, 16
ER = E * R              # 128
TOK = B_ * S            # 8192
NCORES = 8
TPC = TOK // NCORES     # 1024 tokens per core
KT = D // 128           # 32 contraction tiles
NOB = O // 512          # 8 output-column panels
NTT = TPC // 128        # 8 token tiles per core
Q_LO, Q_HI = 611, 2047  # question tokens [611, 2047) within each batch

NG0 = 6                 # token groups held open during the k-outer panel-0 pass
USE_FP8 = True
NK8 = 8                 # trailing k-tiles of the base matmul done in e4m3 DoubleRow
KB = KT - NK8 if USE_FP8 else KT   # bf16 k-tiles in the base matmul
NWARM = 24              # PE warm-up matmuls during the initial DMA window

# x/w panel-0 DMA k-groups (finer first groups so the PE can start sooner)
XGROUPS = [(0, 1), (1, 2), (2, 4), (4, 8), (8, 12), (12, 16), (16, 20), (20, 24), (24, 28), (28, 32)]

_CACHE: dict = {}
LAST_RESULTS = None
TRACE = False


def _build_nc():
    import concourse.bacc as bacc
    import concourse.mybir as mybir
    from concourse import tile

    fp32 = mybir.dt.float32
    fp16 = mybir.dt.float16
    bf16 = mybir.dt.bfloat16
    fp8e4 = mybir.dt.float8e4
    DR = mybir.MatmulPerfMode.DoubleRow

    nc = bacc.Bacc(
        "TRN2",
        target_bir_lowering=False,
        debug=False,
        num_devices=NCORES,
    )

    # all device inputs are pre-swizzled SBUF images (partition-major)
    xswz = nc.dram_tensor("xswz", [128, KT * TPC], bf16, kind="ExternalInput")
    wswz = nc.dram_tensor("wswz", [128, NOB * KB * 512], bf16, kind="ExternalInput")
    afswz = nc.dram_tensor("afswz", [128, KT * ER], bf16, kind="ExternalInput")
    bfT = nc.dram_tensor("bfT", [ER, O], bf16, kind="ExternalInput")
    biasrep = nc.dram_tensor("biasrep", [128, O], bf16, kind="ExternalInput")
    svec = nc.dram_tensor("svec", [128, 1], fp32, kind="ExternalInput")
    if USE_FP8:
        x8swz = nc.dram_tensor("x8swz", [128, NK8 * TPC], fp8e4, kind="ExternalInput")
        w8swz = nc.dram_tensor(
            "w8swz", [128, NOB * NK8 * 512], fp8e4, kind="ExternalInput"
        )
    out = nc.dram_tensor("out", [TPC, O], fp16, kind="ExternalOutput")

    PW = KB * 512           # bf16 panel width in SBUF columns
    PW8 = NK8 * 512         # fp8 panel width

    with tile.TileContext(nc) as tc:
        with (
            tc.tile_pool(name="const", bufs=1) as const,
            tc.tile_pool(name="w", bufs=2) as wpool,
            tc.tile_pool(name="ot", bufs=6) as otpool,
            tc.tile_pool(name="po", bufs=NG0, space="PSUM") as po_pool,
            tc.tile_pool(name="pt", bufs=2, space="PSUM") as pt_pool,
        ):
            # ---- resident SBUF tensors ----
            xt_sb = const.tile([128, KT * TPC], bf16)      # [p, (k t)]
            afT_sb = const.tile([128, KT * ER], bf16)      # [p, (k er)]
            bfT_sb = const.tile([128, O], bf16)            # [er, o]
            biasrep_sb = const.tile([128, O], bf16)
            svec_sb = const.tile([128, 1], fp32)
            u_sb = const.tile([128, TPC], bf16)            # [er, t]
            warm_sb = const.tile([128, 512], bf16)
            if USE_FP8:
                x8_sb2d = const.tile([128, NK8 * TPC], fp8e4)
                x8_sb = x8_sb2d.rearrange("p (k t) -> p k t", k=NK8)  # [p, k8, t]

            w0t = wpool.tile([128, PW], bf16, tag="w", name="w0")

            # ---- PE warm-up: run before any data arrives ----
            nc.any.memset(warm_sb[:], 0)
            warm_ps = pt_pool.tile([128, 512], fp32, tag="pt", name="warm_ps")
            for i in range(NWARM):
                nc.tensor.matmul(
                    warm_ps[:],
                    warm_sb[:, 0:128],
                    warm_sb[:],
                    start=(i == 0),
                    stop=(i == NWARM - 1),
                )

            # ---- priority DMAs, in consumption order ----
            # first x/w k-groups gate the first matmuls
            for (klo, khi) in XGROUPS:
                nc.sync.dma_start(
                    xt_sb[:, klo * TPC:khi * TPC], xswz[:, klo * TPC:khi * TPC]
                )
                if klo < KB:
                    wlo, whi = klo * 512, min(khi, KB) * 512
                    nc.sync.dma_start(w0t[:, wlo:whi], wswz[:, wlo:whi])
                if klo == 0:
                    nc.sync.dma_start(afT_sb[:], afswz[:])
            # needed at the u-scale / first group-close (~55us in)
            nc.sync.dma_start(svec_sb[:], svec[:])
            nc.sync.dma_start(bfT_sb[:], bfT[:])
            nc.sync.dma_start(biasrep_sb[:], biasrep[:])
            if USE_FP8:
                nc.sync.dma_start(x8_sb2d[:], x8swz[:])
                w80t = wpool.tile([128, PW8], fp8e4, tag="w8", name="w80")
                nc.sync.dma_start(w80t[:], w8swz[:, 0:PW8])
            # prefetch panel 1
            w1t = wpool.tile([128, PW], bf16, tag="w", name="w1")
            nc.sync.dma_start(w1t[:], wswz[:, PW:2 * PW])
            if USE_FP8:
                w81t = wpool.tile([128, PW8], fp8e4, tag="w8", name="w81")
                nc.sync.dma_start(w81t[:], w8swz[:, PW8:2 * PW8])

            # ---- panel 0, k-outer: NG0 token groups + LoRA-t groups ----
            pt_tiles = [
                pt_pool.tile([128, 512], fp32, tag="pt", name=f"pt_{i}")
                for i in range(2)
            ]
            po0 = [
                po_pool.tile([128, 512], fp32, tag="po", name=f"po0_{i}")
                for i in range(NG0)
            ]
            for k in range(KT):
                # po MMs first: they only need x+w (afT lands a bit later)
                if k < KB:
                    for tt in range(NG0):
                        nc.tensor.matmul(
                            po0[tt][:],
                            xt_sb[:, k * TPC + tt * 128: k * TPC + tt * 128 + 128],
                            w0t[:, k * 512:(k + 1) * 512],
                            start=(k == 0),
                            stop=False,
                        )
                for tb in range(2):
                    nc.tensor.matmul(
                        pt_tiles[tb][:],
                        afT_sb[:, k * ER:(k + 1) * ER],
                        xt_sb[:, k * TPC + tb * 512: k * TPC + tb * 512 + 512],
                        start=(k == 0),
                        stop=(k == KT - 1),
                    )

            # u = t * routing (per-partition scalar), bf16
            for tb in range(2):
                nc.vector.tensor_scalar_mul(
                    u_sb[:, tb * 512:(tb + 1) * 512],
                    pt_tiles[tb][:],
                    svec_sb[:, 0:1],
                )

            def close_and_drain(po, tt, ob, w8tile):
                if USE_FP8:
                    for j2 in range(0, NK8, 2):
                        nc.tensor.matmul(
                            po[:],
                            x8_sb[:, j2:j2 + 2, tt * 128:(tt + 1) * 128],
                            w8tile[:, j2:j2 + 2, 0:512],
                            start=False,
                            stop=False,
                            perf_mode=DR,
                        )
                nc.tensor.matmul(
                    po[:],
                    u_sb[:, tt * 128:(tt + 1) * 128],
                    bfT_sb[:, ob * 512:(ob + 1) * 512],
                    start=False,
                    stop=True,
                )
                ot = otpool.tile([128, 512], fp16)
                nc.vector.tensor_add(
                    ot[:], po[:], biasrep_sb[:, ob * 512:(ob + 1) * 512]
                )
                nc.sync.dma_start(
                    out[tt * 128:(tt + 1) * 128, ob * 512:(ob + 1) * 512],
                    ot[:],
                )

            w80v = w80t.rearrange("p (k j) -> p k j", k=NK8) if USE_FP8 else None
            for tt in range(NG0):
                close_and_drain(po0[tt], tt, 0, w80v)
            # remaining token tiles of panel 0, tt-outer (x is resident now)
            for tt in range(NG0, NTT):
                po = po_pool.tile([128, 512], fp32, tag="po", name="po")
                for k in range(KB):
                    nc.tensor.matmul(
                        po[:],
                        xt_sb[:, k * TPC + tt * 128: k * TPC + tt * 128 + 128],
                        w0t[:, k * 512:(k + 1) * 512],
                        start=(k == 0),
                        stop=False,
                    )
                close_and_drain(po, tt, 0, w80v)

            # ---- panels 1..7 (double-buffered whole-panel w DMAs) ----
            wt_cur, w8_cur = w1t, (w81t if USE_FP8 else None)
            for ob in range(1, NOB):
                if ob + 1 < NOB:
                    wt_next = wpool.tile(
                        [128, PW], bf16, tag="w", name=f"w{ob + 1}"
                    )
                    nc.sync.dma_start(
                        wt_next[:], wswz[:, (ob + 1) * PW:(ob + 2) * PW]
                    )
                    if USE_FP8:
                        w8_next = wpool.tile(
                            [128, PW8], fp8e4, tag="w8", name=f"w8{ob + 1}"
                        )
                        nc.sync.dma_start(
                            w8_next[:], w8swz[:, (ob + 1) * PW8:(ob + 2) * PW8]
                        )
                    else:
                        w8_next = None
                else:
                    wt_next, w8_next = None, None
                w8v = w8_cur.rearrange("p (k j) -> p k j", k=NK8) if USE_FP8 else None
                for tt in range(NTT):
                    po = po_pool.tile([128, 512], fp32, tag="po", name="po")
                    for k in range(KB):
                        nc.tensor.matmul(
                            po[:],
                            xt_sb[:, k * TPC + tt * 128: k * TPC + tt * 128 + 128],
                            wt_cur[:, k * 512:(k + 1) * 512],
                            start=(k == 0),
                            stop=False,
                        )
                    close_and_drain(po, tt, ob, w8v)
                wt_cur, w8_cur = wt_next, w8_next

    nc.compile()
    return nc


def _host_prep(x, W, b, A, B, router_W, router_b):
    xf = np.ascontiguousarray(x, dtype=np.float32).reshape(TOK, D)
    Wb = W.astype(BF16)
    # per-core SBUF image of x: xswz[p, k*TPC + t] = x[c*TPC + t, k*128 + p]
    xswz_cores, x8_cores = [], []
    for c in range(NCORES):
        xc = xf[c * TPC:(c + 1) * TPC].astype(BF16)          # [TPC, D]
        img = np.ascontiguousarray(
            xc.reshape(TPC, KT, 128).transpose(2, 1, 0)
        ).reshape(128, KT * TPC)
        xswz_cores.append(img)
        if USE_FP8:
            x8_cores.append(
                np.ascontiguousarray(img[:, KB * TPC:]).astype(E4M3)
            )
    # W image: wswz[p, (ob*KB + k)*512 + j] = W[ob*512 + j, k*128 + p], k < KB
    wimg = np.ascontiguousarray(
        Wb.reshape(NOB, 512, KT, 128).transpose(3, 0, 2, 1)
    )                                                        # [128, NOB, KT, 512]
    wswz = np.ascontiguousarray(wimg[:, :, :KB, :]).reshape(128, NOB * KB * 512)
    w8swz = (
        np.ascontiguousarray(wimg[:, :, KB:, :]).astype(E4M3)
        .reshape(128, NOB * NK8 * 512)
        if USE_FP8 else None
    )
    # A image: afswz[p, k*ER + e] = A_flat[e, k*128 + p]
    afswz = np.ascontiguousarray(
        A.reshape(ER, D).astype(BF16).reshape(ER, KT, 128).transpose(2, 1, 0)
    ).reshape(128, KT * ER)
    bfT_bf = (2.0 * np.transpose(B, (0, 2, 1)).reshape(ER, O)).astype(BF16)
    bias_bf = np.ascontiguousarray(
        np.broadcast_to(b.astype(BF16)[None, :], (128, O))
    )
    # router on host (numpy, float64 — exact vs bf16 device noise)
    xq = np.asarray(x, np.float64)[:, Q_LO:Q_HI, :]
    q = xq.mean(axis=1)
    logits = q @ np.asarray(router_W, np.float64).T + np.asarray(router_b, np.float64)
    ex = np.exp(logits - logits.max(-1, keepdims=True))
    routing = ex / ex.sum(-1, keepdims=True)          # [B, E]

    in_maps = []
    for c in range(NCORES):
        sv = np.repeat(routing[c // 2].astype(np.float32), R).reshape(128, 1)
        im = {
            "xswz": xswz_cores[c],
            "wswz": wswz,
            "afswz": afswz,
            "bfT": bfT_bf,
            "biasrep": bias_bf,
            "svec": np.ascontiguousarray(sv),
        }
        if USE_FP8:
            im["x8swz"] = x8_cores[c]
            im["w8swz"] = w8swz
        in_maps.append(im)
    return in_maps


def kernel(x, W, b, A, B, router_W, router_b):
    global LAST_RESULTS
    from concourse.bass_utils import run_bass_kernel_spmd

    if "nc" not in _CACHE:
        _CACHE["nc"] = _build_nc()
    nc = _CACHE["nc"]

    in_maps = _host_prep(x, W, b, A, B, router_W, router_b)

    kwargs = {}
    if TRACE:
        kwargs.update(trace=True, trace_cores=list(range(NCORES)))
    res = run_bass_kernel_spmd(nc, in_maps, core_ids=list(range(NCORES)), **kwargs)
    LAST_RESULTS = res

    shards = [res.results[c]["out"] for c in range(NCORES)]
    return np.concatenate(shards, axis=0).reshape(B_, S, O).astype(np.float32)


# revision 12
# speedup vs baseline: 1.0205x; 1.0205x over previous
# LoRA-MoE QK kernel for 8x Trainium2 NeuronCores (Bass/Tile).
#
# Reference computation:
#   routing = softmax(mean(x[:, 611:-1, :]) @ router_W.T + router_b)   [B, E]
#   base    = x @ W.T + b
#   lora    = einsum('bsd,erd->bser', x, A) -> *B,routing -> [B,S,O] * 2.0
#   out     = base + lora
#
# Sharding: data-parallel over the 8192 tokens (1024/core; each core's tokens
# belong to exactly one batch; a batch spans cores {2b, 2b+1}).  Weights
# replicated, host-prepped; router computed on host.
#
# v6 (530us baseline -> 495us swizzle/startup -> 454us NK8=6 -> 437us):
#  - Host-swizzled SBUF images for all inputs: the DMA engines process one
#    descriptor per partition row (~85ns each), so strided narrow loads are
#    descriptor-rate bound.  Images make every copy 2-32KB-contiguous per
#    row; kernel descriptor count drops ~56k -> ~12k and the first matmul
#    starts at ~11us instead of 38us.
#  - DMA issue order == consumption order; panel 0 is consumed k-outer with
#    6 concurrent token PSUM groups + 2 LoRA-t groups (all 8 banks).
#  - PE warm-up: ~20 dummy matmuls on a zeroed SBUF tile run while the first
#    DMAs land, so the HAM clock-gate is already at 8/8 (2.4GHz) when real
#    matmuls start (saves the ~44-matmul cold ramp, ~6us).
#  - Partial fp8: the last NK8=8 of 32 k-tiles of the base matmul run as
#    e4m3 DoubleRow pairs (4 DR MMs replace 8 bf16 MMs per PSUM group),
#    saving ~37us of PE time.  Error is host-side quantization noise and
#    scales as sqrt(NK8): measured 0.0129/0.0157/0.0181 at NK8=4/6/8 vs
#    the 0.02 gate (deterministic for the fixed test seed).
#  - fp16 output (halves output traffic; values are O(10)).

import numpy as np
import ml_dtypes

BF16 = ml_dtypes.bfloat16
E4M3 = ml_dtypes.float8_e4m3fn

B_, S, D, O, E, R = 4, 2048, 4096, 4096, 8, 16
ER = E * R              # 128
TOK = B_ * S            # 8192
NCORES = 8
TPC = TOK // NCORES     # 1024 tokens per core
KT = D // 128           # 32 contraction tiles
NOB = O // 512          # 8 output-column panels
NTT = TPC // 128        # 8 token tiles per core
Q_LO, Q_HI = 611, 2047  # question tokens [611, 2047) within each batch

NG0 = 6                 # token groups held open during the k-outer panel-0 pass
USE_FP8 = True
NK8 = 10                # trailing k-tiles of the base matmul done in e4m3 DoubleRow
KB = KT - NK8 if USE_FP8 else KT   # bf16 k-tiles in the base matmul
NWARM = 24              # PE warm-up matmuls during the initial DMA window

# x/w panel-0 DMA k-groups (finer first groups so the PE can start sooner)
XGROUPS = [(0, 1), (1, 2), (2, 4), (4, 8), (8, 12), (12, 16), (16, 20), (20, 24), (24, 28), (28, 32)]

_CACHE: dict = {}
LAST_RESULTS = None
TRACE = False


def _build_nc():
    import concourse.bacc as bacc
    import concourse.mybir as mybir
    from concourse import tile

    fp32 = mybir.dt.float32
    fp16 = mybir.dt.float16
    bf16 = mybir.dt.bfloat16
    fp8e4 = mybir.dt.float8e4
    DR = mybir.MatmulPerfMode.DoubleRow

    nc = bacc.Bacc(
        "TRN2",
        target_bir_lowering=False,
        debug=False,
        num_devices=NCORES,
    )

    # all device inputs are pre-swizzled SBUF images (partition-major)
    xswz = nc.dram_tensor("xswz", [128, KT * TPC], bf16, kind="ExternalInput")
    wswz = nc.dram_tensor("wswz", [128, NOB * KB * 512], bf16, kind="ExternalInput")
    afswz = nc.dram_tensor("afswz", [128, KT * ER], bf16, kind="ExternalInput")
    bfT = nc.dram_tensor("bfT", [ER, O], bf16, kind="ExternalInput")
    biasrep = nc.dram_tensor("biasrep", [128, O], bf16, kind="ExternalInput")
    svec = nc.dram_tensor("svec", [128, 2], fp32, kind="ExternalInput")
    if USE_FP8:
        x8swz = nc.dram_tensor("x8swz", [128, NK8 * TPC], fp8e4, kind="ExternalInput")
        w8swz = nc.dram_tensor(
            "w8swz", [128, NOB * NK8 * 512], fp8e4, kind="ExternalInput"
        )
    out = nc.dram_tensor("out", [TPC, O], fp16, kind="ExternalOutput")

    PW = KB * 512           # bf16 panel width in SBUF columns
    PW8 = NK8 * 512         # fp8 panel width

    with tile.TileContext(nc) as tc:
        with (
            tc.tile_pool(name="const", bufs=1) as const,
            tc.tile_pool(name="w", bufs=2) as wpool,
            tc.tile_pool(name="ot", bufs=6) as otpool,
            tc.tile_pool(name="po", bufs=NG0, space="PSUM") as po_pool,
            tc.tile_pool(name="pt", bufs=2, space="PSUM") as pt_pool,
        ):
            # ---- resident SBUF tensors ----
            xt_sb = const.tile([128, KT * TPC], bf16)      # [p, (k t)]
            afT_sb = const.tile([128, KT * ER], bf16)      # [p, (k er)]
            bfT_sb = const.tile([128, O], bf16)            # [er, o]
            biasrep_sb = const.tile([128, O], bf16)
            svec_sb = const.tile([128, 2], fp32)
            u_sb = const.tile([128, TPC], bf16)            # [er, t]
            warm_sb = const.tile([128, 512], bf16)
            if USE_FP8:
                x8_sb2d = const.tile([128, NK8 * TPC], fp8e4)
                x8_sb = x8_sb2d.rearrange("p (k t) -> p k t", k=NK8)  # [p, k8, t]

            w0t = wpool.tile([128, PW], bf16, tag="w", name="w0")

            # ---- PE warm-up: run before any data arrives ----
            nc.any.memset(warm_sb[:], 0)
            warm_ps = pt_pool.tile([128, 512], fp32, tag="pt", name="warm_ps")
            for i in range(NWARM):
                nc.tensor.matmul(
                    warm_ps[:],
                    warm_sb[:, 0:128],
                    warm_sb[:],
                    start=(i == 0),
                    stop=(i == NWARM - 1),
                )

            # ---- priority DMAs, in consumption order ----
            # first x/w k-groups gate the first matmuls
            for (klo, khi) in XGROUPS:
                nc.sync.dma_start(
                    xt_sb[:, klo * TPC:khi * TPC], xswz[:, klo * TPC:khi * TPC]
                )
                if klo < KB:
                    wlo, whi = klo * 512, min(khi, KB) * 512
                    nc.sync.dma_start(w0t[:, wlo:whi], wswz[:, wlo:whi])
                if klo == 0:
                    nc.sync.dma_start(afT_sb[:], afswz[:])
            # needed at the u-scale / first group-close (~55us in)
            nc.sync.dma_start(svec_sb[:], svec[:])
            nc.sync.dma_start(bfT_sb[:], bfT[:])
            nc.sync.dma_start(biasrep_sb[:], biasrep[:])
            if USE_FP8:
                nc.sync.dma_start(x8_sb2d[:], x8swz[:])
                w80t = wpool.tile([128, PW8], fp8e4, tag="w8", name="w80")
                nc.sync.dma_start(w80t[:], w8swz[:, 0:PW8])
            # prefetch panel 1
            w1t = wpool.tile([128, PW], bf16, tag="w", name="w1")
            nc.sync.dma_start(w1t[:], wswz[:, PW:2 * PW])
            if USE_FP8:
                w81t = wpool.tile([128, PW8], fp8e4, tag="w8", name="w81")
                nc.sync.dma_start(w81t[:], w8swz[:, PW8:2 * PW8])

            # ---- panel 0, k-outer: NG0 token groups + LoRA-t groups ----
            pt_tiles = [
                pt_pool.tile([128, 512], fp32, tag="pt", name=f"pt_{i}")
                for i in range(2)
            ]
            po0 = [
                po_pool.tile([128, 512], fp32, tag="po", name=f"po0_{i}")
                for i in range(NG0)
            ]
            for k in range(KT):
                # po MMs first: they only need x+w (afT lands a bit later)
                if k < KB:
                    for tt in range(NG0):
                        nc.tensor.matmul(
                            po0[tt][:],
                            xt_sb[:, k * TPC + tt * 128: k * TPC + tt * 128 + 128],
                            w0t[:, k * 512:(k + 1) * 512],
                            start=(k == 0),
                            stop=False,
                        )
                for tb in range(2):
                    nc.tensor.matmul(
                        pt_tiles[tb][:],
                        afT_sb[:, k * ER:(k + 1) * ER],
                        xt_sb[:, k * TPC + tb * 512: k * TPC + tb * 512 + 512],
                        start=(k == 0),
                        stop=(k == KT - 1),
                    )

            # u = t * routing (per-partition scalar), bf16
            for tb in range(2):
                nc.vector.tensor_scalar_mul(
                    u_sb[:, tb * 512:(tb + 1) * 512],
                    pt_tiles[tb][:],
                    svec_sb[:, 0:1],
                )

            def close_and_drain(po, tt, ob, w8tile):
                if USE_FP8:
                    for j2 in range(0, NK8, 2):
                        nc.tensor.matmul(
                            po[:],
                            x8_sb[:, j2:j2 + 2, tt * 128:(tt + 1) * 128],
                            w8tile[:, j2:j2 + 2, 0:512],
                            start=False,
                            stop=False,
                            perf_mode=DR,
                        )
                nc.tensor.matmul(
                    po[:],
                    u_sb[:, tt * 128:(tt + 1) * 128],
                    bfT_sb[:, ob * 512:(ob + 1) * 512],
                    start=False,
                    stop=True,
                )
                tmp = otpool.tile([128, 512], fp32, tag="tmp", name="tmp")
                nc.vector.tensor_scalar_mul(tmp[:], po[:], svec_sb[:, 1:2])
                ot = otpool.tile([128, 512], fp16)
                nc.vector.tensor_add(
                    ot[:], tmp[:], biasrep_sb[:, ob * 512:(ob + 1) * 512]
                )
                nc.sync.dma_start(
                    out[tt * 128:(tt + 1) * 128, ob * 512:(ob + 1) * 512],
                    ot[:],
                )

            w80v = w80t.rearrange("p (k j) -> p k j", k=NK8) if USE_FP8 else None
            for tt in range(NG0):
                close_and_drain(po0[tt], tt, 0, w80v)
            # remaining token tiles of panel 0, tt-outer (x is resident now)
            for tt in range(NG0, NTT):
                po = po_pool.tile([128, 512], fp32, tag="po", name="po")
                for k in range(KB):
                    nc.tensor.matmul(
                        po[:],
                        xt_sb[:, k * TPC + tt * 128: k * TPC + tt * 128 + 128],
                        w0t[:, k * 512:(k + 1) * 512],
                        start=(k == 0),
                        stop=False,
                    )
                close_and_drain(po, tt, 0, w80v)

            # ---- panels 1..7 (double-buffered whole-panel w DMAs) ----
            wt_cur, w8_cur = w1t, (w81t if USE_FP8 else None)
            for ob in range(1, NOB):
                if ob + 1 < NOB:
                    wt_next = wpool.tile(
                        [128, PW], bf16, tag="w", name=f"w{ob + 1}"
                    )
                    nc.sync.dma_start(
                        wt_next[:], wswz[:, (ob + 1) * PW:(ob + 2) * PW]
                    )
                    if USE_FP8:
                        w8_next = wpool.tile(
                            [128, PW8], fp8e4, tag="w8", name=f"w8{ob + 1}"
                        )
                        nc.sync.dma_start(
                            w8_next[:], w8swz[:, (ob + 1) * PW8:(ob + 2) * PW8]
                        )
                    else:
                        w8_next = None
                else:
                    wt_next, w8_next = None, None
                w8v = w8_cur.rearrange("p (k j) -> p k j", k=NK8) if USE_FP8 else None
                for tt in range(NTT):
                    po = po_pool.tile([128, 512], fp32, tag="po", name="po")
                    for k in range(KB):
                        nc.tensor.matmul(
                            po[:],
                            xt_sb[:, k * TPC + tt * 128: k * TPC + tt * 128 + 128],
                            wt_cur[:, k * 512:(k + 1) * 512],
                            start=(k == 0),
                            stop=False,
                        )
                    close_and_drain(po, tt, ob, w8v)
                wt_cur, w8_cur = wt_next, w8_next

    nc.compile()
    return nc


def _host_prep(x, W, b, A, B, router_W, router_b):
    xf = np.ascontiguousarray(x, dtype=np.float32).reshape(TOK, D)
    Wb = (64.0 * W).astype(BF16)
    # per-core SBUF image of x: xswz[p, k*TPC + t] = x[c*TPC + t, k*128 + p]
    xswz_cores, x8_cores = [], []
    for c in range(NCORES):
        xc = xf[c * TPC:(c + 1) * TPC].astype(BF16)          # [TPC, D]
        img = np.ascontiguousarray(
            xc.reshape(TPC, KT, 128).transpose(2, 1, 0)
        ).reshape(128, KT * TPC)
        xswz_cores.append(img)
        if USE_FP8:
            x8_cores.append(
                np.ascontiguousarray(img[:, KB * TPC:]).astype(E4M3)
            )
    # W image: wswz[p, (ob*KB + k)*512 + j] = W[ob*512 + j, k*128 + p], k < KB
    wimg = np.ascontiguousarray(
        Wb.reshape(NOB, 512, KT, 128).transpose(3, 0, 2, 1)
    )                                                        # [128, NOB, KT, 512]
    wswz = np.ascontiguousarray(wimg[:, :, :KB, :]).reshape(128, NOB * KB * 512)
    w8swz = (
        np.ascontiguousarray(wimg[:, :, KB:, :]).astype(E4M3)
        .reshape(128, NOB * NK8 * 512)
        if USE_FP8 else None
    )
    # A image: afswz[p, k*ER + e] = A_flat[e, k*128 + p]
    afswz = np.ascontiguousarray(
        A.reshape(ER, D).astype(BF16).reshape(ER, KT, 128).transpose(2, 1, 0)
    ).reshape(128, KT * ER)
    bfT_bf = (2.0 * np.transpose(B, (0, 2, 1)).reshape(ER, O)).astype(BF16)
    bias_bf = np.ascontiguousarray(
        np.broadcast_to(b.astype(BF16)[None, :], (128, O))
    )
    # router on host (numpy, float64 — exact vs bf16 device noise)
    xq = np.asarray(x, np.float64)[:, Q_LO:Q_HI, :]
    q = xq.mean(axis=1)
    logits = q @ np.asarray(router_W, np.float64).T + np.asarray(router_b, np.float64)
    ex = np.exp(logits - logits.max(-1, keepdims=True))
    routing = ex / ex.sum(-1, keepdims=True)          # [B, E]

    in_maps = []
    for c in range(NCORES):
        sv = np.stack(
            [
                np.repeat(routing[c // 2].astype(np.float32), R) * 64.0,
                np.full(128, 1.0 / 64.0, np.float32),
            ],
            axis=1,
        )
        im = {
            "xswz": xswz_cores[c],
            "wswz": wswz,
            "afswz": afswz,
            "bfT": bfT_bf,
            "biasrep": bias_bf,
            "svec": np.ascontiguousarray(sv),
        }
        if USE_FP8:
            im["x8swz"] = x8_cores[c]
            im["w8swz"] = w8swz
        in_maps.append(im)
    return in_maps


def kernel(x, W, b, A, B, router_W, router_b):
    global LAST_RESULTS
    from concourse.bass_utils import run_bass_kernel_spmd

    if "nc" not in _CACHE:
        _CACHE["nc"] = _build_nc()
    nc = _CACHE["nc"]

    in_maps = _host_prep(x, W, b, A, B, router_W, router_b)

    kwargs = {}
    if TRACE:
        kwargs.update(trace=True, trace_cores=list(range(NCORES)))
    res = run_bass_kernel_spmd(nc, in_maps, core_ids=list(range(NCORES)), **kwargs)
    LAST_RESULTS = res

    shards = [res.results[c]["out"] for c in range(NCORES)]
    return np.concatenate(shards, axis=0).reshape(B_, S, O).astype(np.float32)
